# revision 1
# baseline (speedup 1.0000x reference)
"""Cross-image contrastive loss on 8 TRN2 NeuronCores.

Strategy (row-parallel over the N=4096 pixel dim, 512 rows per core):
  - The label mask for diff_sum is folded into the matmul contraction:
    augmented K = d + L + 1 = 84 with [Fi; onehot_lab; 1]^T [Fjj; C*onehot_jj; -C],
    so masked logits come out of a single matmul and both row reductions
    (sum_s1 and diff_sum) are fused exp+row-sum on the Scalar engine
    (activation accum_out).
  - bf16 matmul inputs (PE 1 cyc/row vs 4 for f32); f32 PSUM accumulation.
  - The rhs tensor is [128, 8192]: rows 0:84 feed the matmul, rows 96:115
    carry the unscaled label one-hots used for the device-side histogram
    (so everything arrives in one well-shaped DMA stream).
  - Each core emits its partial loss; host sums the 8 partials.
"""

import sys

import numpy as np

sys.path.insert(0, "/opt/trn_rl_repo")

import ml_dtypes

TAU = 0.07
EPS = 1e-4
L = 19
D = 64
N = 4096
NCORES = 8
P = N // NCORES  # 512 rows per core
KA = D + L + 1  # 84 augmented contraction
CMASK = 4.25  # bf16-exact mask magnitude; CMASK/TAU ~ 60.7 in the exponent
PB = P // 128  # 4 partition blocks per core
OH = 96  # base partition of the histogram one-hot rows (32-aligned, > KA)

_compiled = None


def _build():
    from concourse import bacc, mybir, tile

    f32 = mybir.dt.float32
    bf16 = mybir.dt.bfloat16
    Exp = mybir.ActivationFunctionType.Exp
    Ln = mybir.ActivationFunctionType.Ln
    X = mybir.AxisListType.X
    add = mybir.AluOpType.add

    nc = bacc.Bacc("TRN2", target_bir_lowering=False, debug=False)

    lhs_d = nc.dram_tensor("lhs", (KA, P), bf16, kind="ExternalInput")
    lhsP_d = nc.dram_tensor("lhsP", (128, 2 * 128), bf16, kind="ExternalInput")
    oh2_d = nc.dram_tensor("oh2", (L, N), bf16, kind="ExternalInput")
    rhs_d = nc.dram_tensor("rhs", (128, 2 * N), bf16, kind="ExternalInput")
    aux_d = nc.dram_tensor("aux", (128, 2 * PB * D), f32, kind="ExternalInput")
    ohlab_d = nc.dram_tensor("ohlab", (L, P), f32, kind="ExternalInput")
    ones_d = nc.dram_tensor("ones", (128, 1), f32, kind="ExternalInput")
    out_d = nc.dram_tensor("out", (1, 1), f32, kind="ExternalOutput")

    NG = 4  # chunk-pairs (psum groups per p-block), 2048 cols each

    with tile.TileContext(nc) as tc:
        with (
            tc.tile_pool(name="res", bufs=1) as res,
            tc.tile_pool(name="scr", bufs=3) as scr,
            tc.tile_pool(name="ps", bufs=2, space="PSUM") as psp,
        ):
            # preload the act table set that serves BOTH Exp and Ln so the
            # pass doesn't emit a second mid-kernel ACT_TABLE_LOAD
            nc.scalar.add_instruction(
                mybir.InstLoadActFuncSet(
                    name=nc.get_next_instruction_name(),
                    act_func_set_id=6,  # natural_log_exp_and_others
                    ins=[],
                    outs=[],
                )
            )

            # ---- resident SBUF tensors ----
            lhs_sb = res.tile([KA, P], bf16, tag="lhs")
            lhsP_sb = res.tile([128, 2 * 128], bf16, tag="lhsP")
            oh2_sb = res.tile([L, N], bf16, tag="oh2")
            rhs_sb = res.tile([128, 2 * N], bf16, tag="rhs")
            aux_sb = res.tile([128, 2 * PB * D], f32, tag="aux")
            ohlab_sb = res.tile([L, P], f32, tag="ohlab")
            ones_sb = res.tile([128, 1], f32, tag="ones")
            acc = res.tile([128, 16], f32, tag="acc")  # col = cp*4 + b
            zeros = res.tile([128, 1], f32, tag="zeros")
            nc.vector.memset(zeros[:], 0.0)

            for h in range(2):
                nc.sync.dma_start(
                    rhs_sb[:, h * 1024 : (h + 1) * 1024],
                    rhs_d[:, h * 1024 : (h + 1) * 1024],
                )
            nc.sync.dma_start(lhsP_sb[:], lhsP_d[:])
            for h in range(2, 2 * NG):
                nc.sync.dma_start(
                    rhs_sb[:, h * 1024 : (h + 1) * 1024],
                    rhs_d[:, h * 1024 : (h + 1) * 1024],
                )
            nc.sync.dma_start(lhs_sb[:], lhs_d[:])
            nc.sync.dma_start(oh2_sb[:], oh2_d[:])
            nc.sync.dma_start(aux_sb[:], aux_d[:])
            nc.sync.dma_start(ohlab_sb[:], ohlab_d[:])
            nc.sync.dma_start(ones_sb[:], ones_d[:])

            # ---- histograms (bf16-exact 2-stage reduces) ----
            # cnt_ii from oh2 at base 0; cnt_jj from rhs S2 rows at base OH
            part = res.tile([128, 64], f32, tag="part")
            nc.vector.tensor_reduce(
                part[0:L, :],
                oh2_sb[:].rearrange("p (k e) -> p k e", e=64),
                axis=X,
                op=add,
            )
            cnt = res.tile([128, 2], f32, tag="cnt")
            nc.vector.tensor_reduce(
                cnt[0:L, 0:1],
                part[0:L, :].rearrange("p (t k) -> p t k", k=64),
                axis=X,
                op=add,
            )
            partj = res.tile([128, 64], f32, tag="partj")
            nc.vector.tensor_reduce(
                partj[OH : OH + L, :],
                rhs_sb[OH : OH + L, N:].rearrange("p (k e) -> p k e", e=64),
                axis=X,
                op=add,
            )
            cntj = res.tile([128, 1], f32, tag="cntj")
            nc.vector.tensor_reduce(
                cntj[OH : OH + L, :],
                partj[OH : OH + L, :].rearrange("p (t k) -> p t k", k=64),
                axis=X,
                op=add,
            )
            # realign cnt_jj to base 0 next to cnt_ii
            nc.sync.dma_start(cnt[0:L, 1:2], cntj[OH : OH + L, :])
            dn = res.tile([L, 1], f32, tag="dn")
            nc.vector.tensor_add(dn[:], cnt[0:L, 0:1], cnt[0:L, 1:2])
            nc.vector.tensor_scalar_add(dn[:], dn[:], EPS)
            rec = res.tile([L, 1], f32, tag="rec")
            nc.vector.reciprocal(rec[:], dn[:])
            wl = res.tile([L, 1], f32, tag="wl")
            nc.vector.tensor_mul(wl[:], cnt[0:L, 0:1], rec[:])
            # fold -1/N into the weight so the final reduction is the loss
            nc.vector.tensor_scalar_mul(wl[:], wl[:], -1.0 / N)

            # ---- diag = sum_d Fi * (Fii + Fjj), per-64 group sums ----
            prod = res.tile([128, PB * D], f32, tag="prod")
            nc.vector.tensor_mul(
                prod[:], aux_sb[:, 0 : PB * D], aux_sb[:, PB * D : 2 * PB * D]
            )
            dg = res.tile([128, PB], f32, tag="dg")
            nc.vector.tensor_reduce(
                dg[:],
                prod[:].rearrange("p (b e) -> p b e", e=D),
                axis=X,
                op=add,
            )
            t1 = res.tile([128, PB], f32, tag="t1")
            nc.vector.tensor_scalar_mul(t1[:], dg[:], 1.0 / TAU)

            # ---- main S1/S2 pass: matmul -> exp; row-sums on DVE ----
            # S1 (cp 0,1): K=64, two p-blocks packed into PE row groups
            # S2 (cp 2,3): K=84 augmented (mask folded into contraction)
            def finish_group(ps, col):
                dump = scr.tile([128, 2048], bf16, tag="dump")
                nc.scalar.activation(
                    dump[:],
                    ps[:],
                    Exp,
                    bias=zeros[:],
                    scale=1.0 / TAU,
                    accum_out=acc[:, col : col + 1],
                )

            for cp in range(2):
                for bp in range(2):
                    ps_a = psp.tile([128, 2048], f32, tag="mm")
                    ps_b = psp.tile([128, 2048], f32, tag="mm")
                    for c in range(4):
                        cs = slice(cp * 2048 + c * 512, cp * 2048 + (c + 1) * 512)
                        nc.tensor.matmul(
                            ps_a[:, c * 512 : (c + 1) * 512],
                            lhsP_sb[0:64, bp * 128 : (bp + 1) * 128],
                            rhs_sb[0:64, cs],
                            start=True,
                            stop=True,
                            tile_position=(0, 0),
                        )
                        nc.tensor.matmul(
                            ps_b[:, c * 512 : (c + 1) * 512],
                            lhsP_sb[64:128, bp * 128 : (bp + 1) * 128],
                            rhs_sb[64:128, cs],
                            start=True,
                            stop=True,
                            tile_position=(64, 0),
                        )
                    finish_group(ps_a, cp * 4 + 2 * bp)
                    finish_group(ps_b, cp * 4 + 2 * bp + 1)

            for cp in range(2, NG):
                for b in range(PB):
                    ps = psp.tile([128, 2048], f32, tag="mm")
                    for c in range(4):
                        nc.tensor.matmul(
                            ps[:, c * 512 : (c + 1) * 512],
                            lhs_sb[:, b * 128 : (b + 1) * 128],
                            rhs_sb[
                                0:KA, cp * 2048 + c * 512 : cp * 2048 + (c + 1) * 512
                            ],
                            start=True,
                            stop=True,
                        )
                    finish_group(ps, cp * 4 + b)

            # ---- Z = sum of the 4 group-sums per p-block, then logZ ----
            zpm = res.tile([128, PB], f32, tag="zpm")
            nc.vector.tensor_reduce(
                zpm[:],
                acc[:].rearrange("p (g b) -> p b g", b=PB),
                axis=X,
                op=add,
            )
            nc.vector.tensor_scalar_add(zpm[:], zpm[:], EPS)
            logz = res.tile([128, PB], f32, tag="logz")
            nc.scalar.activation(logz[:], zpm[:], Ln, bias=zeros[:])

            # ---- gather weights to partition-major [128, PB] ----
            wps = psp.tile([128, 2048], f32, tag="mm")
            for b in range(PB):
                nc.tensor.matmul(
                    wps[:, b : b + 1],
                    ohlab_sb[:, b * 128 : (b + 1) * 128],
                    wl[:],
                    start=True,
                    stop=True,
                )
            w_pm = res.tile([128, PB], f32, tag="wpm")
            nc.vector.tensor_copy(w_pm[:], wps[:, 0:PB])

            # ---- values = w * (diag/tau - 2*logZ); partial = sum ----
            vals = res.tile([128, PB], f32, tag="vals")
            nc.vector.scalar_tensor_tensor(
                out=vals[:],
                in0=logz[:],
                scalar=-2.0,
                in1=t1[:],
                op0=mybir.AluOpType.mult,
                op1=add,
            )
            nc.vector.tensor_mul(vals[:], vals[:], w_pm[:])
            vred = res.tile([128, 1], f32, tag="vred")
            nc.vector.tensor_reduce(vred[:], vals[:], axis=X, op=add)

            fin = psp.tile([128, 2048], f32, tag="mm")
            nc.tensor.matmul(
                fin[0:1, 0:1], ones_sb[:], vred[:], start=True, stop=True
            )
            res_sb = res.tile([1, 1], f32, tag="res")
            nc.scalar.copy(res_sb[:], fin[0:1, 0:1])
            nc.sync.dma_start(out_d[:], res_sb[:])

    nc.compile()
    return nc


def _make_in_maps(features_i, features_ii, features_jj, i, ii, jj):
    bf16 = ml_dtypes.bfloat16
    Fi = features_i.reshape(D, N).astype(np.float32)
    Fii = features_ii.reshape(D, N).astype(np.float32)
    Fjj = features_jj.reshape(D, N).astype(np.float32)
    lab = i.reshape(-1)
    ii_f = ii.reshape(-1)
    jj_f = jj.reshape(-1)

    lids = np.arange(L, dtype=np.int32)
    oh_jj = (jj_f[None, :] == lids[:, None]).astype(np.float32)  # [L, N]
    oh_ii = (ii_f[None, :] == lids[:, None]).astype(np.float32)

    # rhs (replicated): [128, 2N] = [S1 | S2]; histogram rows at OH
    rhs = np.zeros((128, 2 * N), np.float32)
    rhs[0:D, 0:N] = Fii
    rhs[D : 2 * D, 0:N] = Fii  # duplicate for the row-packed S1 pair
    rhs[0:D, N:] = Fjj
    rhs[D : D + L, N:] = CMASK * oh_jj
    rhs[D + L, N:] = -CMASK
    rhs[OH : OH + L, N:] = oh_jj
    rhs = rhs.astype(bf16)
    oh2 = oh_ii.astype(bf16)

    ones = np.ones((128, 1), np.float32)

    in_maps = []
    for c in range(NCORES):
        sel = slice(c * P, (c + 1) * P)
        lab_c = lab[sel]
        lhs = np.zeros((KA, P), np.float32)
        lhs[0:D] = Fi[:, sel]
        lhs[D : D + L] = (lab_c[None, :] == lids[:, None]).astype(np.float32)
        lhs[D + L] = 1.0

        # partition-major transposed feature blocks: aux = [fiT | fsT]
        aux = np.zeros((128, 2 * PB * D), np.float32)
        Fsum = Fii[:, sel] + Fjj[:, sel]
        Fic = Fi[:, sel]
        for b in range(PB):
            blk = slice(b * 128, (b + 1) * 128)
            aux[:, b * D : (b + 1) * D] = Fic[:, blk].T
            aux[:, PB * D + b * D : PB * D + (b + 1) * D] = Fsum[:, blk].T

        ohlab = (lab_c[None, :] == lids[:, None]).astype(np.float32)  # [L, P]

        lhsP = np.zeros((128, 2 * 128), np.float32)
        for bp in range(2):
            lhsP[0:D, bp * 128 : (bp + 1) * 128] = Fic[:, 2 * bp * 128 : (2 * bp + 1) * 128]
            lhsP[D : 2 * D, bp * 128 : (bp + 1) * 128] = Fic[:, (2 * bp + 1) * 128 : (2 * bp + 2) * 128]

        in_maps.append(
            {
                "lhs": lhs.astype(bf16),
                "lhsP": lhsP.astype(bf16),
                "oh2": oh2,
                "rhs": rhs,
                "aux": aux,
                "ohlab": ohlab,
                "ones": ones,
            }
        )
    return in_maps


_LDW_PATCHED = False


def _enable_ldw_opt():
    """Flip walrus --enable-ldw-opt for this process (dedups back-to-back
    LDWEIGHTS of the same stationary operand)."""
    global _LDW_PATCHED
    if _LDW_PATCHED:
        return
    from concourse import bass_utils

    orig = bass_utils.run_command

    def patched(cmd, *a, **kw):
        if isinstance(cmd, list):
            cmd = [
                "--enable-ldw-opt=true" if c == "--enable-ldw-opt=false" else c
                for c in cmd
            ]
        return orig(cmd, *a, **kw)

    bass_utils.run_command = patched
    _LDW_PATCHED = True


def kernel(features_i, features_ii, features_jj, i, ii, jj):
    global _compiled
    from concourse import bass_utils

    if _compiled is None:
        _compiled = _build()
    in_maps = _make_in_maps(features_i, features_ii, features_jj, i, ii, jj)
    results = bass_utils.run_bass_kernel_spmd(
        _compiled, in_maps, core_ids=list(range(NCORES))
    )
    total = np.float32(0.0)
    for r in results.results:
        total += np.float32(r["out"].reshape(-1)[0])
    return np.array(total, dtype=np.float32)



# revision 3
# speedup vs baseline: 1.1542x; 1.1542x over previous
"""Cross-image contrastive loss on 8 TRN2 NeuronCores — v2.

Key ideas over the v1 baseline (52.8us):
  - Host-side precompute of everything O(N): label histograms, weights
    w = cnt_ii/(denom+eps), the diagonal term sum_p w*(diag1+diag2)/tau.
    The device only computes Z_p = sum_s1 + diff_sum and returns
    sum_p (2/N) w_p log(Z_p+eps); the host adds the constant.
  - Label-sorted blocking for S2: rows (pixels of image i) are sorted by
    label i, columns (pixels of jj) sorted by label jj. diff_sum[p] only
    involves columns with jj==lab_p, so each 128-row chunk needs a single
    contiguous 1024-column window instead of all 4096 columns (the CMASK
    fold inside the K=84 augmented contraction still kills wrong-label /
    padding columns).  S2 exp work drops 4x.
  - The exp+row-sum work is split across TWO engines: the Scalar/ACT
    engine (native Exp activation with accum_out) and the Vector/DVE
    engine via two custom DVE ops implementing exp by repeated squaring:
      OP1: t = P(x*c0)^8,  P(v) = 1 + v + v^2/2   (deg-2 Taylor)
      OP2: out = t^64, accum_out = row-sum
    so exp(l/tau) ~= P(l/(512*tau))^512 (rel err ~2e-3, fine vs 2e-2).
  - Unpaired K=64 S1 matmuls (no duplicated Fii rows -> 0.5MB less DMA),
    fine-grained DMA issue order so compute starts at ~2-3us.
"""

import sys

import numpy as np

sys.path.insert(0, "/opt/trn_rl_repo")

import ml_dtypes

TAU = 0.07
EPS = 1e-4
L = 19
D = 64
N = 4096
NCORES = 8
P = N // NCORES  # 512 rows per core
KA = D + L + 1  # 84 augmented contraction for S2
CMASK = 4.25  # bf16-exact mask magnitude; CMASK/TAU ~ 60.7 in the exponent
W2 = 1024  # S2 window columns per 128-row chunk
C0V = 1.0 / (TAU * 512.0)  # DVE exp: x*C0V -> ^512

_compiled = None
_EXPA8 = None
_SQ64SUM = None


def _ref_expa8(in0, in1, c0, c1, c2):
    x = np.asarray(in0, np.float32)
    c0 = np.float32(c0) if not isinstance(c0, np.ndarray) else c0
    c1 = np.float32(c1) if not isinstance(c1, np.ndarray) else c1
    v = (x * c0).astype(np.float32)
    p = ((1.0 + v) + (v * v) * c1).astype(np.float32)
    for _ in range(3):
        p = (p * p).astype(np.float32)
    return p


def _ref_sq64(in0, in1, c0, c1, c2):
    t = np.asarray(in0, np.float32)
    for _ in range(6):
        t = (t * t).astype(np.float32)
    return t, t.sum(axis=-1, keepdims=True)


def _register_dve_ops():
    """Register the two exp-by-repeated-squaring custom DVE ops (idempotent)."""
    global _EXPA8, _SQ64SUM
    if _EXPA8 is not None:
        return
    from concourse import dve_ops
    from concourse.dve_spec import AluOp, C1, One, Spec, Src0, lower, sq
    from concourse.dve_spec import _has_src1 as has_src1
    from concourse.dve_uop import DveOpSpec

    def mk(name, spec):
        for op in dve_ops.OPS:
            if op.name == name:
                return op
        row = dve_ops._CUSTOM_DVE_ROW_BASE + len(dve_ops.OPS)
        tmp = DveOpSpec(
            name=name, opcode=row, uops=lower(spec, ver="v3"), rd1_en=has_src1(spec)
        )
        op = dve_ops.DveOp(name, spec, subdim=False, uops_sha={"v3": tmp.sha("v3")})
        dve_ops.OPS.append(op)
        dve_ops._SUB_OPCODE_FOR_NAME[name] = row
        dve_ops.CUSTOM_DVE_SPECS[name] = spec
        return op

    from concourse.dve_spec import C0 as C0leaf

    v = Src0 * C0leaf
    body1 = (One + v) + sq(v) * C1  # 1 + v + v^2/2 with C1=0.5
    spec1 = Spec(body=sq(sq(sq(body1))), reference=_ref_expa8)
    spec2 = Spec(
        body=sq(sq(sq(sq(sq(sq(Src0)))))), accum=AluOp.ADD, reference=_ref_sq64
    )
    _EXPA8 = mk("ANT_EXPA8", spec1)
    _SQ64SUM = mk("ANT_SQ64SUM", spec2)


# unit assignment per core: 4 chunks x (S1 h0, S1 h1, S2 window)
# ACT: S1 (0,0) (0,1) (1,0) (1,1) (2,0); S2 w0 w1 w2
# DVE: S1 (2,1) (3,0) (3,1); S2 w3
DVE_S1 = {(2, 1), (3, 0), (3, 1)}
DVE_S2 = {3}


def _build():
    from concourse import bacc, mybir, tile

    _register_dve_ops()

    f32 = mybir.dt.float32
    bf16 = mybir.dt.bfloat16
    Exp = mybir.ActivationFunctionType.Exp
    Ln = mybir.ActivationFunctionType.Ln
    X = mybir.AxisListType.X
    add = mybir.AluOpType.add

    nc = bacc.Bacc("TRN2", target_bir_lowering=False, debug=False)

    lhs1_d = nc.dram_tensor("lhs1", (D, P), bf16, kind="ExternalInput")
    lhs2_d = nc.dram_tensor("lhs2", (KA, P), bf16, kind="ExternalInput")
    rhs1_d = nc.dram_tensor("rhs1", (D, N), bf16, kind="ExternalInput")
    rhs2_d = nc.dram_tensor("rhs2", (KA, 4 * W2), bf16, kind="ExternalInput")
    wz_d = nc.dram_tensor("wz", (128, 4), f32, kind="ExternalInput")
    out_d = nc.dram_tensor("out", (1, 1), f32, kind="ExternalOutput")

    with tile.TileContext(nc) as tc:
        with (
            tc.tile_pool(name="res", bufs=1) as res,
            tc.tile_pool(name="tsc", bufs=2) as tsc,
            tc.tile_pool(name="ps", bufs=2, space="PSUM") as psp,
        ):
            # preload the table set that serves both Exp and Ln
            nc.scalar.add_instruction(
                mybir.InstLoadActFuncSet(
                    name=nc.get_next_instruction_name(),
                    act_func_set_id=6,  # natural_log_exp_and_others
                    ins=[],
                    outs=[],
                )
            )

            # ---- resident SBUF tensors ----
            lhs1_sb = res.tile([D, P], bf16, tag="lhs1")
            lhs2_sb = res.tile([KA, P], bf16, tag="lhs2")
            rhs1_sb = res.tile([D, N], bf16, tag="rhs1")
            rhs2_sb = res.tile([KA, 4 * W2], bf16, tag="rhs2")
            wz_sb = res.tile([128, 4], f32, tag="wz")
            accA = res.tile([128, 16], f32, tag="accA")
            accD = res.tile([128, 16], f32, tag="accD")
            dumpA = res.tile([128, 2048], bf16, tag="dumpA")
            junkD = res.tile([128, 2048], bf16, tag="junkD")
            zeros = res.tile([128, 1], f32, tag="zeros")
            ones = res.tile([128, 1], f32, tag="ones")

            nc.vector.memset(zeros[:], 0.0)
            nc.vector.memset(accA[:], 0.0)
            nc.gpsimd.memset(accD[:], 0.0)
            nc.gpsimd.memset(ones[:], 1.0)

            # ---- DMAs, critical-first ----
            nc.sync.dma_start(lhs1_sb[:], lhs1_d[:])
            nc.sync.dma_start(lhs2_sb[:], lhs2_d[:])
            nc.sync.dma_start(rhs1_sb[:, 0:2048], rhs1_d[:, 0:2048])
            nc.sync.dma_start(rhs1_sb[:, 2048:4096], rhs1_d[:, 2048:4096])
            nc.sync.dma_start(rhs2_sb[:], rhs2_d[:])
            nc.sync.dma_start(wz_sb[:], wz_d[:])

            def fill_s1(b, h):
                ps = psp.tile([128, 2048], f32, tag="mm")
                for c in range(4):
                    cs = slice(h * 2048 + c * 512, h * 2048 + (c + 1) * 512)
                    nc.tensor.matmul(
                        ps[:, c * 512 : (c + 1) * 512],
                        lhs1_sb[:, b * 128 : (b + 1) * 128],
                        rhs1_sb[:, cs],
                        start=True,
                        stop=True,
                    )
                return ps

            def fill_s2_half(ps, half, b):
                for c in range(2):
                    nc.tensor.matmul(
                        ps[:, half * W2 + c * 512 : half * W2 + (c + 1) * 512],
                        lhs2_sb[:, b * 128 : (b + 1) * 128],
                        rhs2_sb[:, b * W2 + c * 512 : b * W2 + (c + 1) * 512],
                        start=True,
                        stop=True,
                    )

            def act_unit(src, col, width):
                nc.scalar.activation(
                    dumpA[:, 0:width],
                    src,
                    Exp,
                    bias=zeros[:],
                    scale=1.0 / TAU,
                    accum_out=accA[:, col : col + 1],
                )

            def dve_unit(src, col, width):
                t = tsc.tile([128, 2048], f32, tag="t")
                nc.vector._custom_dve(
                    _EXPA8, out=t[:, 0:width], in0=src, s0=C0V, s1=0.5
                )
                nc.vector._custom_dve(
                    _SQ64SUM,
                    out=junkD[:, 0:width],
                    in0=t[:, 0:width],
                    accum_out=accD[:, col : col + 1],
                )

            def consume(ps_slice, b, j, width):
                col = 4 * b + j
                is_dve = (
                    ((b, j) in DVE_S1) if j < 2 else (b in DVE_S2)
                )
                if is_dve:
                    dve_unit(ps_slice, col, width)
                else:
                    act_unit(ps_slice, col, width)

            # ---- S1 units, DVE-first interleave ----
            order = [(3, 0), (0, 0), (3, 1), (0, 1), (2, 1), (1, 0), (1, 1), (2, 0)]
            for b, h in order:
                ps = fill_s1(b, h)
                consume(ps[:], b, h, 2048)

            # ---- S2 windows: two psum groups, two windows each ----
            ps = psp.tile([128, 2048], f32, tag="mm")
            fill_s2_half(ps, 0, 3)  # DVE window first
            fill_s2_half(ps, 1, 0)
            consume(ps[:, 0:W2], 3, 2, W2)
            consume(ps[:, W2 : 2 * W2], 0, 2, W2)
            ps = psp.tile([128, 2048], f32, tag="mm")
            fill_s2_half(ps, 0, 1)
            fill_s2_half(ps, 1, 2)
            consume(ps[:, 0:W2], 1, 2, W2)
            consume(ps[:, W2 : 2 * W2], 2, 2, W2)

            # ---- Z = sum over the 4 accum columns of each chunk ----
            zA = res.tile([128, 4], f32, tag="zA")
            zD = res.tile([128, 4], f32, tag="zD")
            nc.vector.tensor_reduce(
                zA[:], accA[:].rearrange("p (b j) -> p b j", j=4), axis=X, op=add
            )
            nc.vector.tensor_reduce(
                zD[:], accD[:].rearrange("p (b j) -> p b j", j=4), axis=X, op=add
            )
            zsum = res.tile([128, 4], f32, tag="zsum")
            nc.vector.tensor_add(zsum[:], zA[:], zD[:])
            nc.vector.tensor_scalar_add(zsum[:], zsum[:], EPS)
            logz = res.tile([128, 4], f32, tag="logz")
            nc.scalar.activation(logz[:], zsum[:], Ln, bias=zeros[:])

            # ---- partial = sum_p wz * logz ----
            vals = res.tile([128, 4], f32, tag="vals")
            nc.vector.tensor_mul(vals[:], logz[:], wz_sb[:])
            vred = res.tile([128, 1], f32, tag="vred")
            nc.vector.tensor_reduce(vred[:], vals[:], axis=X, op=add)
            fin = psp.tile([128, 2048], f32, tag="mm")
            nc.tensor.matmul(fin[0:1, 0:1], ones[:], vred[:], start=True, stop=True)
            res_sb = res.tile([1, 1], f32, tag="res")
            nc.scalar.copy(res_sb[:], fin[0:1, 0:1])
            nc.sync.dma_start(out_d[:], res_sb[:])

    nc.compile()
    return nc


def _make_in_maps(features_i, features_ii, features_jj, i, ii, jj):
    """Host prep. Returns (in_maps, host_const)."""
    bf16 = ml_dtypes.bfloat16
    Fi = features_i.reshape(D, N).astype(np.float32)
    Fii = features_ii.reshape(D, N).astype(np.float32)
    Fjj = features_jj.reshape(D, N).astype(np.float32)
    lab = i.reshape(-1).astype(np.int64)
    ii_f = ii.reshape(-1).astype(np.int64)
    jj_f = jj.reshape(-1).astype(np.int64)

    cnt_ii = np.bincount(ii_f, minlength=L).astype(np.float32)
    cnt_jj = np.bincount(jj_f, minlength=L).astype(np.float32)
    wl = cnt_ii / (cnt_ii + cnt_jj + np.float32(EPS))
    w = wl[lab]  # [N] original order
    diag = (Fi * (Fii + Fjj)).sum(axis=0) / np.float32(TAU)
    host_const = np.float32(-(w @ diag) / N)

    perm = np.argsort(lab, kind="stable")
    sFi = Fi[:, perm]
    slab = lab[perm]
    sw = w[perm]

    cperm = np.argsort(jj_f, kind="stable")
    sFjj = Fjj[:, cperm]
    sjj = jj_f[cperm]
    seg = np.searchsorted(sjj, np.arange(L + 1))  # seg[l] = first col of label l

    # global augmented S2 matrix [KA, N + W2] (padding cols are killed by
    # the constant -CMASK row with all-zero one-hots)
    G = np.zeros((KA, N + W2), np.float32)
    G[0:D, 0:N] = sFjj
    G[D + sjj, np.arange(N)] = CMASK
    G[D + L, :] = -CMASK

    woff = []
    for g in range(N // 128):
        la, lb = slab[g * 128], slab[g * 128 + 127]
        off, end = seg[la], seg[lb + 1]
        assert end - off <= W2, f"S2 window overflow: chunk {g}: {end - off}"
        woff.append(int(off))

    rhs1 = Fii.astype(bf16)  # [64, N], unsorted columns

    in_maps = []
    for c in range(NCORES):
        sel = slice(c * P, (c + 1) * P)
        lhs1 = sFi[:, sel]
        sl = slab[sel]
        lhs2 = np.zeros((KA, P), np.float32)
        lhs2[0:D] = lhs1
        lhs2[D + sl, np.arange(P)] = 1.0
        lhs2[D + L] = 1.0
        rhs2 = np.concatenate(
            [G[:, woff[4 * c + b] : woff[4 * c + b] + W2] for b in range(4)], axis=1
        )
        wz = (np.float32(2.0) / N) * sw[sel].reshape(4, 128).T.copy()
        in_maps.append(
            {
                "lhs1": lhs1.astype(bf16),
                "lhs2": lhs2.astype(bf16),
                "rhs1": rhs1,
                "rhs2": rhs2.astype(bf16),
                "wz": wz.astype(np.float32),
            }
        )
    return in_maps, host_const


def kernel(features_i, features_ii, features_jj, i, ii, jj):
    global _compiled
    from concourse import bass_utils

    if _compiled is None:
        _compiled = _build()
    in_maps, host_const = _make_in_maps(
        features_i, features_ii, features_jj, i, ii, jj
    )
    results = bass_utils.run_bass_kernel_spmd(
        _compiled, in_maps, core_ids=list(range(NCORES))
    )
    total = np.float32(host_const)
    for r in results.results:
        total += np.float32(r["out"].reshape(-1)[0])
    return np.array(total, dtype=np.float32)


# revision 4
# speedup vs baseline: 1.3958x; 1.2093x over previous
"""Cross-image contrastive loss on 8 TRN2 NeuronCores — v3.

v2 -> v3: the exp+rowsum section was pipeline-stalled (each engine ~50%
idle) because ACT units (4 PSUM banks) and DVE units (4 banks) plus a
PE fill could not coexist in the 8-bank PSUM.  v3 partitions PSUM
6 banks / 2 banks:
  - ACT: two 1536-col groups (3 banks each) -> double-buffered; S1
    chunk-rows are consumed as 1536+1536+1024 activations.
  - DVE: one 1024-col group; pass 2 of the custom exp runs off-PSUM so
    the next fill overlaps it.
Also: S2 windows shrink to 768 cols, and the assignment is rebalanced
(ACT ~16.8us busy, DVE ~16.6us, PE ~14us at the observed 1.2GHz).

Algorithm (see v2 notes): host precomputes histograms/weights/diagonal;
rows and S2 columns label-sorted so diff_sum needs only a 768-col
window per 128-row chunk (CMASK fold in the K=84 augmented contraction
kills wrong-label and padding columns); device returns
sum_p (2/N) w_p log(Z_p + eps) per core; exp is split between the ACT
Exp activation (accum_out) and two custom DVE ops computing
exp(l/tau) ~= P(l/(512 tau))^512 by repeated squaring.
"""

import sys

import numpy as np

sys.path.insert(0, "/opt/trn_rl_repo")

import ml_dtypes

TAU = 0.07
EPS = 1e-4
L = 19
D = 64
N = 4096
NCORES = 8
P = N // NCORES  # 512 rows per core
KA = D + L + 1  # 84 augmented contraction for S2
CMASK = 4.25
W2 = 768  # S2 window columns per 128-row chunk
C0V = 1.0 / (TAU * 512.0)

_compiled = None
_EXPA8 = None
_SQ64SUM = None


def _ref_expa8(in0, in1, c0, c1, c2):
    x = np.asarray(in0, np.float32)
    c0 = np.float32(c0) if not isinstance(c0, np.ndarray) else c0
    c1 = np.float32(c1) if not isinstance(c1, np.ndarray) else c1
    v = (x * c0).astype(np.float32)
    p = ((1.0 + v) + (v * v) * c1).astype(np.float32)
    for _ in range(3):
        p = (p * p).astype(np.float32)
    return p


def _ref_sq64(in0, in1, c0, c1, c2):
    t = np.asarray(in0, np.float32)
    for _ in range(6):
        t = (t * t).astype(np.float32)
    return t, t.sum(axis=-1, keepdims=True)


def _register_dve_ops():
    global _EXPA8, _SQ64SUM
    if _EXPA8 is not None:
        return
    from concourse import dve_ops
    from concourse.dve_spec import AluOp, C1, One, Spec, Src0, lower, sq
    from concourse.dve_spec import C0 as C0leaf
    from concourse.dve_spec import _has_src1 as has_src1
    from concourse.dve_uop import DveOpSpec

    def mk(name, spec):
        for op in dve_ops.OPS:
            if op.name == name:
                return op
        row = dve_ops._CUSTOM_DVE_ROW_BASE + len(dve_ops.OPS)
        tmp = DveOpSpec(
            name=name, opcode=row, uops=lower(spec, ver="v3"), rd1_en=has_src1(spec)
        )
        op = dve_ops.DveOp(name, spec, subdim=False, uops_sha={"v3": tmp.sha("v3")})
        dve_ops.OPS.append(op)
        dve_ops._SUB_OPCODE_FOR_NAME[name] = row
        dve_ops.CUSTOM_DVE_SPECS[name] = spec
        return op

    v = Src0 * C0leaf
    body1 = (One + v) + sq(v) * C1  # 1 + v + v^2/2 with C1=0.5
    spec1 = Spec(body=sq(sq(sq(body1))), reference=_ref_expa8)
    spec2 = Spec(
        body=sq(sq(sq(sq(sq(sq(Src0)))))), accum=AluOp.ADD, reference=_ref_sq64
    )
    _EXPA8 = mk("ANT_EXPA8", spec1)
    _SQ64SUM = mk("ANT_SQ64SUM", spec2)


# ---- static unit schedule ---------------------------------------------------
# Per core: 4 chunks (128 rows each).  S1 = 4096 cols vs rhs1; S2 = one
# W2-col window vs rhs2.  Units: ("A"|"D", kind, chunk, col_off, width, jslot)
# jslot indexes the accumulator column: acc[:, 6*chunk + jslot].
# ACT: S1 of ch0, ch1 fully; ch2 cols 2048:4096; S2 w0,w1,w2.
# DVE: S1 ch2 cols 0:2048 (2x1024), ch3 fully (4x1024); S2 w3.
def _unit_schedule():
    A, Dv = [], []
    for b in (0, 1):
        A.append(("s1", b, 0, 1536, 0))
        A.append(("s1", b, 1536, 1536, 1))
        A.append(("s1", b, 3072, 1024, 2))
    A.append(("s1", 2, 2048, 1536, 2))
    A.append(("s1", 2, 3584, 512, 3))
    for b in (0, 1, 2):
        A.append(("s2", b, 0, W2, 5))
    for q in range(2):
        Dv.append(("s1", 2, q * 1024, 1024, q))
    for q in range(4):
        Dv.append(("s1", 3, q * 1024, 1024, q))
    Dv.append(("s2", 3, 0, W2, 5))
    return A, Dv


def _build():
    from concourse import bacc, mybir, tile

    _register_dve_ops()

    f32 = mybir.dt.float32
    bf16 = mybir.dt.bfloat16
    Exp = mybir.ActivationFunctionType.Exp
    Ln = mybir.ActivationFunctionType.Ln
    X = mybir.AxisListType.X
    add = mybir.AluOpType.add

    nc = bacc.Bacc("TRN2", target_bir_lowering=False, debug=False)

    lhs1_d = nc.dram_tensor("lhs1", (D, P), bf16, kind="ExternalInput")
    lhs2_d = nc.dram_tensor("lhs2", (KA, P), bf16, kind="ExternalInput")
    rhs1_d = nc.dram_tensor("rhs1", (D, N), bf16, kind="ExternalInput")
    rhs2_d = nc.dram_tensor("rhs2", (KA, 4 * W2), bf16, kind="ExternalInput")
    wz_d = nc.dram_tensor("wz", (128, 4), f32, kind="ExternalInput")
    out_d = nc.dram_tensor("out", (1, 1), f32, kind="ExternalOutput")

    with tile.TileContext(nc) as tc:
        with (
            tc.tile_pool(name="res", bufs=1) as res,
            tc.tile_pool(name="tsc", bufs=2) as tsc,
            tc.tile_pool(name="psA", bufs=2, space="PSUM") as psA,
            tc.tile_pool(name="psD", bufs=1, space="PSUM") as psD,
        ):
            nc.scalar.add_instruction(
                mybir.InstLoadActFuncSet(
                    name=nc.get_next_instruction_name(),
                    act_func_set_id=6,
                    ins=[],
                    outs=[],
                )
            )

            lhs1_sb = res.tile([D, P], bf16, tag="lhs1")
            lhs2_sb = res.tile([KA, P], bf16, tag="lhs2")
            rhs1_sb = res.tile([D, N], bf16, tag="rhs1")
            rhs2_sb = res.tile([KA, 4 * W2], bf16, tag="rhs2")
            wz_sb = res.tile([128, 4], f32, tag="wz")
            accA = res.tile([128, 24], f32, tag="accA")
            accD = res.tile([128, 24], f32, tag="accD")
            dumpA = res.tile([128, 1536], bf16, tag="dumpA")
            junkD = res.tile([128, 1024], bf16, tag="junkD")
            zeros = res.tile([128, 1], f32, tag="zeros")
            ones = res.tile([128, 1], f32, tag="ones")

            nc.vector.memset(zeros[:], 0.0)
            nc.vector.memset(accA[:], 0.0)
            nc.gpsimd.memset(accD[:], 0.0)
            nc.gpsimd.memset(ones[:], 1.0)

            nc.sync.dma_start(lhs1_sb[:], lhs1_d[:])
            nc.sync.dma_start(lhs2_sb[:], lhs2_d[:])
            nc.sync.dma_start(rhs1_sb[:, 0:2048], rhs1_d[:, 0:2048])
            nc.sync.dma_start(rhs1_sb[:, 2048:4096], rhs1_d[:, 2048:4096])
            nc.sync.dma_start(rhs2_sb[:], rhs2_d[:])
            nc.sync.dma_start(wz_sb[:], wz_d[:])

            def fill(ps, kind, b, off, width):
                """Matmuls for unit (kind,b,off,width) into ps[:, 0:width]."""
                done = 0
                while done < width:
                    step = min(512, width - done)
                    if kind == "s1":
                        nc.tensor.matmul(
                            ps[:, done : done + step],
                            lhs1_sb[:, b * 128 : (b + 1) * 128],
                            rhs1_sb[:, off + done : off + done + step],
                            start=True,
                            stop=True,
                        )
                    else:
                        nc.tensor.matmul(
                            ps[:, done : done + step],
                            lhs2_sb[:, b * 128 : (b + 1) * 128],
                            rhs2_sb[:, b * W2 + done : b * W2 + done + step],
                            start=True,
                            stop=True,
                        )
                    done += step

            def act_unit(u):
                kind, b, off, width, j = u
                ps = psA.tile([128, 1536], f32, tag="mmA")
                fill(ps, kind, b, off, width)
                nc.scalar.activation(
                    dumpA[:, 0:width],
                    ps[:, 0:width],
                    Exp,
                    bias=zeros[:],
                    scale=1.0 / TAU,
                    accum_out=accA[:, 6 * b + j : 6 * b + j + 1],
                )

            def dve_unit(u):
                kind, b, off, width, j = u
                ps = psD.tile([128, 1024], f32, tag="mmD")
                fill(ps, kind, b, off, width)
                t = tsc.tile([128, 1024], f32, tag="t")
                nc.vector._custom_dve(
                    _EXPA8, out=t[:, 0:width], in0=ps[:, 0:width], s0=C0V, s1=0.5
                )
                nc.vector._custom_dve(
                    _SQ64SUM,
                    out=junkD[:, 0:width],
                    in0=t[:, 0:width],
                    accum_out=accD[:, 6 * b + j : 6 * b + j + 1],
                )

            A, Dv = _unit_schedule()
            # interleave: DVE units are the longer queue; lead with them
            seq = []
            ai, di = 0, 0
            pattern = "DADADADADADAAAAAAA"  # 7 D, 11 A
            for chix in pattern:
                if chix == "D" and di < len(Dv):
                    seq.append(("D", Dv[di]))
                    di += 1
                elif ai < len(A):
                    seq.append(("A", A[ai]))
                    ai += 1
            while di < len(Dv):
                seq.append(("D", Dv[di]))
                di += 1
            while ai < len(A):
                seq.append(("A", A[ai]))
                ai += 1
            for eng, u in seq:
                if eng == "D":
                    dve_unit(u)
                else:
                    act_unit(u)

            # ---- Z, logZ, partial ----
            zA = res.tile([128, 4], f32, tag="zA")
            zD = res.tile([128, 4], f32, tag="zD")
            nc.vector.tensor_reduce(
                zA[:], accA[:].rearrange("p (b j) -> p b j", j=6), axis=X, op=add
            )
            nc.vector.tensor_reduce(
                zD[:], accD[:].rearrange("p (b j) -> p b j", j=6), axis=X, op=add
            )
            zsum = res.tile([128, 4], f32, tag="zsum")
            nc.vector.tensor_add(zsum[:], zA[:], zD[:])
            nc.vector.tensor_scalar_add(zsum[:], zsum[:], EPS)
            logz = res.tile([128, 4], f32, tag="logz")
            nc.scalar.activation(logz[:], zsum[:], Ln, bias=zeros[:])

            vals = res.tile([128, 4], f32, tag="vals")
            nc.vector.tensor_mul(vals[:], logz[:], wz_sb[:])
            vred = res.tile([128, 1], f32, tag="vred")
            nc.vector.tensor_reduce(vred[:], vals[:], axis=X, op=add)
            fin = psD.tile([128, 1024], f32, tag="mmD")
            nc.tensor.matmul(fin[0:1, 0:1], ones[:], vred[:], start=True, stop=True)
            res_sb = res.tile([1, 1], f32, tag="res")
            nc.scalar.copy(res_sb[:], fin[0:1, 0:1])
            nc.sync.dma_start(out_d[:], res_sb[:])

    nc.compile()
    return nc


def _make_in_maps(features_i, features_ii, features_jj, i, ii, jj):
    """Host prep. Returns (in_maps, host_const)."""
    bf16 = ml_dtypes.bfloat16
    Fi = features_i.reshape(D, N).astype(np.float32)
    Fii = features_ii.reshape(D, N).astype(np.float32)
    Fjj = features_jj.reshape(D, N).astype(np.float32)
    lab = i.reshape(-1).astype(np.int64)
    ii_f = ii.reshape(-1).astype(np.int64)
    jj_f = jj.reshape(-1).astype(np.int64)

    cnt_ii = np.bincount(ii_f, minlength=L).astype(np.float32)
    cnt_jj = np.bincount(jj_f, minlength=L).astype(np.float32)
    wl = cnt_ii / (cnt_ii + cnt_jj + np.float32(EPS))
    w = wl[lab]
    diag = (Fi * (Fii + Fjj)).sum(axis=0) / np.float32(TAU)
    host_const = np.float32(-(w @ diag) / N)

    perm = np.argsort(lab, kind="stable")
    sFi = Fi[:, perm]
    slab = lab[perm]
    sw = w[perm]

    cperm = np.argsort(jj_f, kind="stable")
    sFjj = Fjj[:, cperm]
    sjj = jj_f[cperm]
    seg = np.searchsorted(sjj, np.arange(L + 1))

    G = np.zeros((KA, N + W2), np.float32)
    G[0:D, 0:N] = sFjj
    G[D + sjj, np.arange(N)] = CMASK
    G[D + L, :] = -CMASK

    woff = []
    for g in range(N // 128):
        la, lb = slab[g * 128], slab[g * 128 + 127]
        off, end = seg[la], seg[lb + 1]
        assert end - off <= W2, f"S2 window overflow: chunk {g}: {end - off}"
        woff.append(int(off))

    rhs1 = Fii.astype(bf16)

    in_maps = []
    for c in range(NCORES):
        sel = slice(c * P, (c + 1) * P)
        lhs1 = sFi[:, sel]
        sl = slab[sel]
        lhs2 = np.zeros((KA, P), np.float32)
        lhs2[0:D] = lhs1
        lhs2[D + sl, np.arange(P)] = 1.0
        lhs2[D + L] = 1.0
        rhs2 = np.concatenate(
            [G[:, woff[4 * c + b] : woff[4 * c + b] + W2] for b in range(4)], axis=1
        )
        wz = (np.float32(2.0) / N) * sw[sel].reshape(4, 128).T.copy()
        in_maps.append(
            {
                "lhs1": lhs1.astype(bf16),
                "lhs2": lhs2.astype(bf16),
                "rhs1": rhs1,
                "rhs2": rhs2.astype(bf16),
                "wz": wz.astype(np.float32),
            }
        )
    return in_maps, host_const


def kernel(features_i, features_ii, features_jj, i, ii, jj):
    global _compiled
    from concourse import bass_utils

    if _compiled is None:
        _compiled = _build()
    in_maps, host_const = _make_in_maps(
        features_i, features_ii, features_jj, i, ii, jj
    )
    results = bass_utils.run_bass_kernel_spmd(
        _compiled, in_maps, core_ids=list(range(NCORES))
    )
    total = np.float32(host_const)
    for r in results.results:
        total += np.float32(r["out"].reshape(-1)[0])
    return np.array(total, dtype=np.float32)
